# revision 1
# baseline (speedup 1.0000x reference)
"""DocumentCrossAttentionMHA Trainium2 kernel.

Data-parallel over batch: each of the 8 NeuronCores computes one batch
element end-to-end (QKV projections, 8-head cross attention over S=256
sentence vectors with length masking, out-projection, residual,
LayerNorm, mean over the L=2048 query positions).

Dataflow is fully "transposed" ([feature, seq] layouts) so that every
matmul's operands are produced directly by the previous stage with no
on-device transposes:
  qT[d,l]  = WqT.T @ QinT        kT[d,s] = WkT.T @ sentT
  v[s,d]   = sentT.T @ WvT
  eT[s,l]  = exp(kT_h.T @ qT_h + mask[s])          (mask fused in ACT bias)
  den[h,l] = sum_s eT  (selector matmuls)          rec = 1/den
  ctxT[d,l]= (v_h.T @ eT_h) * rec_bcast
  xT[d,l]  = (WoT.T @ ctxT + bo') + QinT     (bo' = bo + Wo@bv, ACT bias;
                                              v-bias folds into bo because
                                              softmax rows sum to 1)
LayerNorm + mean over l collapse to:
  out[d] = ln_w[d]/L * (sum_l xT[d,l]*r[l] - sum_l mu[l]*r[l]) + ln_b[d]
with mu/var per column from ones-matmul partition reductions and the
weighted sums as per-block DVE mult+reduce partials combined at the end.

Hardware landmines discovered on TRN2 (these crash the exec unit with
NRT_EXEC_UNIT_UNRECOVERABLE at runtime despite compiling cleanly):
  - K=1 (single-partition-contraction) fp32 matmuls; broadcasts go
    through DRAM round-trip DMA instead.
  - tensor_tensor_reduce; use tensor_tensor + tensor_reduce pairs.
  - float32r operands (even with compute-op "rounding producers" the
    verifier demands); bf16 is the reliable full-rate PE path.
"""

import time
from contextlib import ExitStack

import numpy as np

import concourse.bacc as bacc
import concourse.bass as bass
import concourse.mybir as mybir
import concourse.tile as tile
from concourse.bass_utils import run_bass_kernel_spmd

B, S, KTOP, D, H = 8, 256, 8, 512, 8
HD = D // H          # 64
L = S * KTOP         # 2048
P = 128
NCH = D // P         # 4 feature chunks
SC = S // P          # 2 s chunks
LBLK = 512
NBLK = L // LBLK     # 4 l blocks
NEG = -1.0e9
F32 = mybir.dt.float32


def build_kernel(dt=mybir.dt.bfloat16, stage=99):
    """stage: 1=kv-proj only, 2=+qproj, 3=+scores/exp, 4=+den/rec,
    5=+ctx, 6=+outproj/xt, 7=+stats, 99=full"""
    """Emit the single-core program (run SPMD on all 8 cores)."""
    nc = bacc.Bacc(trn_type="TRN2", debug=False)
    AF = mybir.ActivationFunctionType
    OP = mybir.AluOpType
    R32 = mybir.dt.float32r

    def mm(out, lhsT, rhs, **kw):
        nc.tensor.matmul(out, lhsT, rhs, **kw)

    def din(name, shape):
        return nc.dram_tensor(name, shape, dt, kind="ExternalInput").ap()

    def din32(name, shape):
        return nc.dram_tensor(name, shape, F32, kind="ExternalInput").ap()

    qin_d = din("qin_t", [P, NCH, L])
    sent_d = din("sent_t", [P, NCH, S])
    wq_d = din("wq", [P, NCH, D])
    wk_d = din("wk", [P, NCH, D])
    wv_d = din("wv", [P, NCH, D])
    wo_d = din("wo", [P, NCH, D])
    bq_d = din32("bq_col", [P, NCH])
    bk_d = din32("bk_col", [P, NCH])
    bo_d = din32("bo_col", [P, NCH])
    ones1p_d = din("ones_1p", [1, P])
    onesc_d = din("ones_col", [P, 1])
    sel8_d = din("sel8", [P, H, H])
    sel2_d = din("sel2", [H, NCH, P])
    mask_d = din32("mask_col", [P, SC])
    lnw_d = din32("lnw_col", [P, NCH])
    lnb_d = din32("lnb_col", [P, NCH])
    ones1pf_d = din32("ones_1pf", [1, P])
    out_d = nc.dram_tensor("out", [D], F32, kind="ExternalOutput").ap()
    scr_r_d = nc.dram_tensor("scr_r", [NBLK, LBLK], F32).ap()
    scr_b_d = nc.dram_tensor("scr_b", [1], F32).ap()

    with tile.TileContext(nc) as tc, ExitStack() as ctx:
        const = ctx.enter_context(tc.tile_pool(name="const", bufs=1))
        ps = ctx.enter_context(tc.tile_pool(name="ps", bufs=5, space="PSUM"))
        psd = ctx.enter_context(tc.tile_pool(name="psd", bufs=1, space="PSUM"))
        psm = ctx.enter_context(tc.tile_pool(name="psm", bufs=1, space="PSUM"))

        def cload(ap_d, shape, dtt):
            t = const.tile(shape, dtt, tag=ap_d.tensor.name)
            nc.sync.dma_start(out=t, in_=ap_d)
            return t

        # loads ordered by first consumption: kv-proj needs sent/wk
        # first, q-proj needs wq + the block-0 qin, wo is needed last
        sent = cload(sent_d, [P, NCH, S], dt)
        wk = cload(wk_d, [P, NCH, D], dt)
        wv = cload(wv_d, [P, NCH, D], dt)
        bk = cload(bk_d, [P, NCH], F32)
        wq = cload(wq_d, [P, NCH, D], dt)
        bq = cload(bq_d, [P, NCH], F32)
        sel8 = cload(sel8_d, [P, H, H], dt)
        sel2 = cload(sel2_d, [H, NCH, P], dt)
        mask = cload(mask_d, [P, SC], F32)
        wo = cload(wo_d, [P, NCH, D], dt)
        bo = cload(bo_d, [P, NCH], F32)
        ones1p = cload(ones1p_d, [1, P], dt)
        onesc = cload(onesc_d, [P, 1], dt)
        lnw = cload(lnw_d, [P, NCH], F32)
        lnb = cload(lnb_d, [P, NCH], F32)
        ones1pf = cload(ones1pf_d, [1, P], F32)

        # ---- k/v projections (once per core) ----
        kt = const.tile([P, NCH, S], dt, tag="kt")
        vsb = const.tile([P, SC, D], dt, tag="vsb")
        for c in range(NCH):
            pk = ps.tile([P, S], F32, tag="mm")
            for kc in range(NCH):
                mm(
                    pk, wk[:, kc, c * P:(c + 1) * P], sent[:, kc, :],
                    start=(kc == 0), stop=(kc == NCH - 1))
            nc.scalar.activation(
                out=kt[:, c, :], in_=pk, func=AF.Identity,
                bias=bk[:, c:c + 1], scale=1.0)
        for sc in range(SC):
            pv = ps.tile([P, D], F32, tag="mm")
            for kc in range(NCH):
                mm(
                    pv, sent[:, kc, sc * P:(sc + 1) * P], wv[:, kc, :],
                    start=(kc == 0), stop=(kc == NCH - 1))
            nc.vector.tensor_copy(out=vsb[:, sc, :], in_=pv)

        blkp = ctx.enter_context(tc.tile_pool(name="blk", bufs=3))
        expp = ctx.enter_context(tc.tile_pool(name="expp", bufs=2))
        stat = ctx.enter_context(tc.tile_pool(name="stat", bufs=2))
        scr = ctx.enter_context(tc.tile_pool(name="scr", bufs=3))
        accp = ctx.enter_context(tc.tile_pool(name="acc", bufs=1))

        bscal = accp.tile([1, 1], F32, tag="bscal")
        asb = accp.tile([P, NCH], F32, tag="asb")
        asb_p = accp.tile([P, NCH, NBLK], F32, tag="asb_p")
        bsc_p = accp.tile([1, NBLK], F32, tag="bsc_p")
        eps_t = accp.tile([1, 1], F32, tag="eps")
        nc.vector.memset(eps_t, 1e-5)

        # ---- main loop over l blocks ----
        for blk in range(NBLK):
            lsl = slice(blk * LBLK, (blk + 1) * LBLK)
            qin = blkp.tile([P, NCH, LBLK], dt, tag="qin")
            nc.sync.dma_start(out=qin, in_=qin_d[:, :, lsl])

            if stage < 2:
                continue
            qt = blkp.tile([P, NCH, LBLK], dt, tag="qt")
            for c in range(NCH):
                pq = ps.tile([P, LBLK], F32, tag="mm")
                for kc in range(NCH):
                    mm(
                        pq, wq[:, kc, c * P:(c + 1) * P], qin[:, kc, :],
                        start=(kc == 0), stop=(kc == NCH - 1))
                nc.scalar.activation(
                    out=qt[:, c, :], in_=pq, func=AF.Identity,
                    bias=bq[:, c:c + 1], scale=1.0)

            if stage < 3:
                continue
            # scores^T + fused mask/exp; chunk index = h*SC + sc
            et = expp.tile([P, H * SC, LBLK], dt, tag="exp")
            for h in range(H):
                pp = (h % 2) * HD
                for sc in range(SC):
                    psc = ps.tile([P, LBLK], F32, tag="mm")
                    mm(
                        psc,
                        kt[pp:pp + HD, h // 2, sc * P:(sc + 1) * P],
                        qt[pp:pp + HD, h // 2, :],
                        start=True, stop=True)
                    nc.scalar.activation(
                        out=et[:, h * SC + sc, :], in_=psc, func=AF.Exp,
                        bias=mask[:, sc:sc + 1], scale=1.0)

            if stage < 4:
                continue
            # denominators for all heads -> [H, LBLK]
            pden = psd.tile([H, LBLK], F32, tag="den")
            n = 0
            for h in range(H):
                for sc in range(SC):
                    mm(
                        pden, sel8[:, h, :], et[:, h * SC + sc, :],
                        start=(n == 0), stop=(n == H * SC - 1))
                    n += 1
            rec_b = stat.tile([H, LBLK], dt, tag="rec_s")
            with nc.allow_low_precision(reason="feeds PE broadcast only"):
                nc.vector.reciprocal(rec_b, pden)

            if stage < 5:
                continue
            # ctx^T scaled by 1/den
            ctxs = blkp.tile([P, NCH, LBLK], dt, tag="ctxs")
            for c in range(NCH):
                prb = ps.tile([P, LBLK], F32, tag="mm")
                mm(prb, sel2[:, c, :], rec_b, start=True, stop=True)
                rb = scr.tile([P, LBLK], F32, tag="rb")
                nc.scalar.copy(rb, prb)
                pca = ps.tile([HD, LBLK], F32, tag="mm")
                pcb = ps.tile([HD, LBLK], F32, tag="mm")
                for sc in range(SC):
                    mm(
                        pca, vsb[:, sc, c * P:c * P + HD],
                        et[:, (2 * c) * SC + sc, :],
                        start=(sc == 0), stop=(sc == SC - 1))
                    mm(
                        pcb, vsb[:, sc, c * P + HD:(c + 1) * P],
                        et[:, (2 * c + 1) * SC + sc, :],
                        start=(sc == 0), stop=(sc == SC - 1))
                nc.vector.tensor_tensor(
                    out=ctxs[0:HD, c, :], in0=pca, in1=rb[0:HD, :],
                    op=OP.mult)
                nc.vector.tensor_tensor(
                    out=ctxs[HD:P, c, :], in0=pcb, in1=rb[HD:P, :],
                    op=OP.mult)

            if stage < 6:
                continue
            # out-projection + bias + residual; LN partial sums
            xt = blkp.tile([P, NCH, LBLK], dt, tag="xt")
            pmu = psm.tile([1, LBLK], F32, tag="mu")
            psq = psm.tile([1, LBLK], F32, tag="sq")
            for e in range(NCH):
                po = ps.tile([P, LBLK], F32, tag="mm")
                for kc in range(NCH):
                    mm(
                        po, wo[:, kc, e * P:(e + 1) * P], ctxs[:, kc, :],
                        start=(kc == 0), stop=(kc == NCH - 1))
                po2 = scr.tile([P, LBLK], dt, tag="po2")
                nc.scalar.activation(
                    out=po2, in_=po, func=AF.Identity,
                    bias=bo[:, e:e + 1], scale=1.0)
                nc.vector.tensor_tensor(
                    out=xt[:, e, :], in0=qin[:, e, :], in1=po2, op=OP.add)
                mm(
                    pmu, onesc, xt[:, e, :],
                    start=(e == 0), stop=(e == NCH - 1))
                x2 = scr.tile([P, LBLK], dt, tag="x2")
                nc.vector.tensor_tensor(
                    out=x2, in0=xt[:, e, :], in1=xt[:, e, :], op=OP.mult)
                mm(
                    psq, onesc, x2,
                    start=(e == 0), stop=(e == NCH - 1))

            if stage < 7:
                continue
            # per-column stats -> r[l]  (ones_col carries 1/D upstream)
            mu = stat.tile([1, LBLK], F32, tag="mu_s")
            nc.vector.tensor_copy(out=mu, in_=pmu)
            mu2 = stat.tile([1, LBLK], F32, tag="mu2_s")
            nc.vector.tensor_tensor(out=mu2, in0=mu, in1=mu, op=OP.mult)
            var = stat.tile([1, LBLK], F32, tag="var_s")
            nc.vector.tensor_tensor(out=var, in0=psq, in1=mu2, op=OP.subtract)
            sd = stat.tile([1, LBLK], F32, tag="sd_s")
            nc.scalar.activation(
                out=sd, in_=var, func=AF.Sqrt, bias=eps_t, scale=1.0)
            r_ = stat.tile([1, LBLK], F32, tag="r_s")
            with nc.allow_low_precision(reason="f32r feeds PE only"):
                nc.vector.reciprocal(r_, sd)

            # per-block partials: bsc_p[blk] = sum_l mu*r ;
            # asb_p[:, e, blk] = sum_l xt*r_bcast
            s1 = scr.tile([1, LBLK], F32, tag="s1")
            nc.vector.tensor_tensor(out=s1, in0=mu, in1=r_, op=OP.mult)
            nc.vector.tensor_reduce(
                out=bsc_p[:, blk:blk + 1], in_=s1,
                axis=mybir.AxisListType.X, op=OP.add)
            nc.sync.dma_start(out=scr_r_d[blk], in_=r_)
            prs = scr.tile([P, LBLK], F32, tag="prs")
            nc.sync.dma_start(
                out=prs,
                in_=bass.AP(tensor=scr_r_d.tensor, offset=blk * LBLK,
                            ap=[[0, P], [1, LBLK]]))
            # one 3D op over all chunks; prs broadcast along the chunk
            # dim via a zero-stride free AP
            prs_b = bass.AP(tensor=prs.tensor, offset=prs.offset,
                            ap=[prs.ap[0], [0, NCH], prs.ap[1]])
            s2 = scr.tile([P, NCH, LBLK], F32, tag="s2")
            nc.vector.tensor_tensor(out=s2, in0=xt, in1=prs_b, op=OP.mult)
            nc.vector.tensor_reduce(
                out=asb_p[:, :, blk], in_=s2,
                axis=mybir.AxisListType.X, op=OP.add)

        # ---- final combine ----
        nc.vector.tensor_reduce(
            out=bscal, in_=bsc_p, axis=mybir.AxisListType.X, op=OP.add)
        nc.vector.tensor_reduce(
            out=asb, in_=asb_p, axis=mybir.AxisListType.X, op=OP.add)
        nc.sync.dma_start(out=scr_b_d, in_=bscal)
        pb = accp.tile([P, 1], F32, tag="pb")
        nc.sync.dma_start(
            out=pb,
            in_=bass.AP(tensor=scr_b_d.tensor, offset=0,
                        ap=[[0, P], [1, 1]]))
        osb = accp.tile([P, NCH], F32, tag="osb")
        t0 = accp.tile([P, NCH], F32, tag="t0")
        pb_b = bass.AP(tensor=pb.tensor, offset=pb.offset,
                       ap=[pb.ap[0], [0, NCH]])
        nc.vector.tensor_tensor(out=t0, in0=asb, in1=pb_b, op=OP.subtract)
        nc.vector.tensor_tensor(out=t0, in0=t0, in1=lnw, op=OP.mult)
        nc.vector.tensor_tensor(out=osb, in0=t0, in1=lnb, op=OP.add)
        od = out_d.rearrange("(c p) -> p c", p=P)
        nc.sync.dma_start(out=od, in_=osb)

    nc.compile()
    return nc


def make_inputs(top_word_vecs, sent_vecs, num_sents, Wq, bq, Wk, bk, Wv, bv,
                Wo, bo, ln_w, ln_b, np_dt=None):
    if np_dt is None:
        import ml_dtypes
        np_dt = ml_dtypes.bfloat16
    """Host-side prep: transposes/layouts + per-core sharding over batch."""
    f32 = np.float32
    scale = 1.0 / np.sqrt(HD)

    def wcol(W):  # [dout, din] -> lhsT layout [128, 4 din-chunks, 512 dout]
        wt = np.ascontiguousarray(W.T.astype(f32))  # [din, dout]
        return wt.reshape(NCH, P, D).transpose(1, 0, 2).astype(np_dt)

    def col(v):  # [512] -> [128, 4]
        return np.ascontiguousarray(v.astype(f32).reshape(NCH, P).T)

    shared = {
        "wq": wcol(np.asarray(Wq) * scale),
        "wk": wcol(np.asarray(Wk)),
        "wv": wcol(np.asarray(Wv)),
        "wo": wcol(np.asarray(Wo)),
        "bq_col": col(np.asarray(bq) * scale),
        "bk_col": col(np.asarray(bk)),
                "bo_col": col(np.asarray(bo, f32) + np.asarray(Wo, f32) @ np.asarray(bv, f32)),
                "ones_1p": np.ones((1, P), np_dt),
        "ones_col": np.full((P, 1), 1.0 / D, np.float32).astype(np_dt),
        "lnw_col": col(np.asarray(ln_w) / L),
        "lnb_col": col(np.asarray(ln_b)),
        "ones_1pf": np.ones((1, P), np.float32),
    }
    sel8 = np.zeros((P, H, H), f32)
    for h in range(H):
        sel8[:, h, h] = 1.0
    shared["sel8"] = sel8.astype(np_dt)
    sel2 = np.zeros((H, NCH, P), f32)
    for c in range(NCH):
        sel2[2 * c, c, 0:HD] = 1.0
        sel2[2 * c + 1, c, HD:P] = 1.0
    shared["sel2"] = sel2.astype(np_dt)

    twv = np.asarray(top_word_vecs, f32).reshape(B, L, D)
    sv = np.asarray(sent_vecs, f32)
    ns = np.asarray(num_sents).astype(np.int64)
    in_maps = []
    for b in range(B):
        qin_t = np.ascontiguousarray(twv[b].T).reshape(NCH, P, L)
        qin_t = qin_t.transpose(1, 0, 2).astype(np_dt)
        sent_t = np.ascontiguousarray(sv[b].T).reshape(NCH, P, S)
        sent_t = sent_t.transpose(1, 0, 2).astype(np_dt)
        mask = np.zeros((P, SC), f32)
        sidx = np.arange(S).reshape(SC, P).T  # [p, sc] -> s
        mask[sidx >= ns[b]] = NEG
        m = dict(shared)
        m["qin_t"] = np.ascontiguousarray(qin_t)
        m["sent_t"] = np.ascontiguousarray(sent_t)
        m["mask_col"] = mask
        in_maps.append(m)
    return in_maps


_NC_CACHE = {}


def _get_nc():
    key = "f32"
    if key not in _NC_CACHE:
        _NC_CACHE[key] = build_kernel()
    return _NC_CACHE[key]


def kernel(**inputs):
    nc = _get_nc()
    in_maps = make_inputs(**inputs)
    res = run_bass_kernel_spmd(nc, in_maps, list(range(B)))
    out = np.stack([res.results[i]["out"] for i in range(B)]).astype(np.float32)
    return out


def _make_sharded(nc, in_maps):
    """Replicate bass2jax.run_bass_via_pjrt's jit/shard_map wiring but
    return a callable over pre-placed device arrays for repeat timing."""
    import jax
    import concourse.mybir as mb
    from concourse import bass2jax
    from jax.sharding import Mesh, PartitionSpec, NamedSharding
    from jax.experimental.shard_map import shard_map

    bass2jax.install_neuronx_cc_hook()
    pid_name = nc.partition_id_tensor.name if nc.partition_id_tensor else None
    in_names, out_names, out_avals = [], [], []
    for alloc in nc.m.functions[0].allocations:
        if not isinstance(alloc, mb.MemoryLocationSet):
            continue
        name = alloc.memorylocations[0].name
        if alloc.kind == "ExternalInput":
            if name != pid_name:
                in_names.append(name)
        elif alloc.kind == "ExternalOutput":
            out_names.append(name)
            out_avals.append(
                jax.core.ShapedArray(tuple(alloc.tensor_shape),
                                     mb.dt.np(alloc.dtype)))
    n_params = len(in_names)
    all_names = in_names + out_names

    def _body(*args):
        operands = list(args)
        if pid_name is not None:
            operands.append(bass2jax.partition_id_tensor())
        outs = bass2jax._bass_exec_p.bind(
            *operands,
            out_avals=tuple(out_avals),
            in_names=tuple(all_names + ([pid_name] if pid_name else [])),
            out_names=tuple(out_names),
            lowering_input_output_aliases=(),
            sim_require_finite=True,
            sim_require_nnan=True,
            nc=nc,
        )
        return tuple(outs)

    devices = jax.devices()[:B]
    mesh = Mesh(np.asarray(devices), ("core",))
    spec = PartitionSpec("core")
    nouts = len(out_names)
    sharded = jax.jit(
        shard_map(_body, mesh=mesh, in_specs=(spec,) * (n_params + nouts),
                  out_specs=(spec,) * nouts, check_rep=False),
        keep_unused=True)
    sh = NamedSharding(mesh, spec)
    args = []
    for i, name in enumerate(in_names):
        cat = np.concatenate([np.asarray(m[name]) for m in in_maps], axis=0)
        args.append(jax.device_put(cat, sh))
    for av in out_avals:
        z = np.zeros((B * av.shape[0], *av.shape[1:]), av.dtype)
        args.append(jax.device_put(z, sh))
    return sharded, args, out_names, out_avals


def bench(n_iters=30, **inputs):
    import jax
    nc = _get_nc()
    in_maps = make_inputs(**inputs)
    sharded, args, out_names, out_avals = _make_sharded(nc, in_maps)
    out = sharded(*args)
    jax.block_until_ready(out)
    t0 = time.perf_counter()
    for _ in range(n_iters):
        out = sharded(*args)
    jax.block_until_ready(out)
    t1 = time.perf_counter()
    return (t1 - t0) / n_iters * 1e9






# revision 3
# speedup vs baseline: 1.5226x; 1.5226x over previous
"""DocumentCrossAttentionMHA Trainium2 kernel, v2.

Data-parallel over batch: each of the 8 NeuronCores computes one batch
element end-to-end.  Same transposed dataflow as v1 (every matmul
consumes its producer's layout directly), plus:

  - bscal (= sum_l mu_l r_l) is derived from asb (sum_d asb[d] = D*bscal),
    so the epilogue needs no mu*r reduction and no DRAM round-trips;
    partition broadcasts ride tiny PE matmuls against an 8-row ones
    selector (rstack trick) instead of DRAM round-trip DMAs.
  - rsqrt for LayerNorm as r = exp(-0.5*ln(var+eps)).  Ln still forces
    an activation-table swap pair per block, but the stats stage is
    software-pipelined one block behind the main loop so the swaps
    overlap PE-heavy phases instead of stalling the softmax exp stream.
  - Block-0 q-projection is emitted before the k/v projections: engine
    queues issue in order, and wq+qin0 arrive from HBM well before
    sent+wk, so the PE starts ~3us earlier.
  - s2 reduction as a bf16 halving tree (tensor_reduce has no 2x DVE
    mode; tensor_tensor does).
  - mu/sq share one PSUM bank (rows 0/32); ctx head pairs share one
    bank ([0:64)+[64:128)) via partition-offset matmul outputs.
  - Inputs packed into 5 tensors (was 18) to cut dispatch/DMA overhead.

Hardware landmines (from v1, crash NRT at runtime if violated):
  - no single-partition-contraction (K=1) fp32 matmuls
  - no tensor_tensor_reduce custom DVE op
  - no float32r operands
"""

import time
from contextlib import ExitStack

import numpy as np

import concourse.bacc as bacc
import concourse.bass as bass
import concourse.mybir as mybir
import concourse.tile as tile
from concourse.bass_utils import run_bass_kernel_spmd

B, S, KTOP, D, H = 8, 256, 8, 512, 8
HD = D // H          # 64
L = S * KTOP         # 2048
P = 128
NCH = D // P         # 4 feature chunks
SC = S // P          # 2 s chunks
LBLK = 512
NBLK = L // LBLK     # 4 l blocks
NEG = -1.0e9
F32 = mybir.dt.float32
BF16 = mybir.dt.bfloat16

# blobW column offsets (bf16 [P, NCH, WCOLS]): sent | wk | wq | wv | wo
W_SENT, W_WK, W_WQ, W_WV, W_WO = 0, S, S + D, S + 2 * D, S + 3 * D
WCOLS = S + 4 * D    # 2304
# blobC column offsets (f32 [P, CCOLS]): bq | bk | bo' | mask | lnw | lnb
C_BQ, C_BK, C_BO, C_MASK = 0, NCH, 2 * NCH, 3 * NCH
C_LNW, C_LNB = 3 * NCH + SC, 4 * NCH + SC
CCOLS = 5 * NCH + SC  # 22
# blobS (bf16 [8, SCOLS]): sel2 flat | onesrow8
S_SEL2, S_ONES8 = 0, NCH * P
SCOLS = NCH * P + P  # 640
# blob2 (bf16 [P, 65]): sel8 flat | onesc (1/D)
B2_SEL8, B2_ONESC = 0, H * H
B2COLS = H * H + 1


def build_kernel():
    nc = bacc.Bacc(trn_type="TRN2", debug=False)
    AF = mybir.ActivationFunctionType
    OP = mybir.AluOpType

    def mm(out, lhsT, rhs, **kw):
        nc.tensor.matmul(out, lhsT, rhs, **kw)

    qin_d = nc.dram_tensor("qin_t", [P, NCH, L], BF16, kind="ExternalInput").ap()
    blobw_d = nc.dram_tensor("blobw", [P, NCH, WCOLS], BF16, kind="ExternalInput").ap()
    blob2_d = nc.dram_tensor("blob2", [P, B2COLS], BF16, kind="ExternalInput").ap()
    blobs_d = nc.dram_tensor("blobs", [H, SCOLS], BF16, kind="ExternalInput").ap()
    blobc_d = nc.dram_tensor("blobc", [P, CCOLS], F32, kind="ExternalInput").ap()
    asbp_d = nc.dram_tensor("asbp", [P, NBLK, NCH], F32, kind="ExternalOutput").ap()

    with tile.TileContext(nc) as tc, ExitStack() as ctx:
        const = ctx.enter_context(tc.tile_pool(name="const", bufs=1))
        blkp = ctx.enter_context(tc.tile_pool(name="blk", bufs=3))
        ps = ctx.enter_context(tc.tile_pool(name="ps", bufs=4, space="PSUM"))
        pctx = ctx.enter_context(tc.tile_pool(name="pctx", bufs=2, space="PSUM"))
        psd = ctx.enter_context(tc.tile_pool(name="psd", bufs=1, space="PSUM"))
        psm = ctx.enter_context(tc.tile_pool(name="psm", bufs=1, space="PSUM"))

        # ---- constant loads, ordered by first consumption ----
        blobw = const.tile([P, NCH, WCOLS], BF16, tag="blobw")
        qin0 = blkp.tile([P, NCH, LBLK], BF16, tag="qin")
        nc.sync.dma_start(out=blobw[:, :, W_WQ:W_WV], in_=blobw_d[:, :, W_WQ:W_WV])
        nc.sync.dma_start(out=qin0, in_=qin_d[:, :, 0:LBLK])
        blobc = const.tile([P, CCOLS], F32, tag="blobc")
        nc.sync.dma_start(out=blobc, in_=blobc_d)
        nc.sync.dma_start(out=blobw[:, :, 0:W_WQ], in_=blobw_d[:, :, 0:W_WQ])
        blob2 = const.tile([P, B2COLS], BF16, tag="blob2")
        nc.sync.dma_start(out=blob2, in_=blob2_d)
        blobs = const.tile([H, SCOLS], BF16, tag="blobs")
        nc.sync.dma_start(out=blobs, in_=blobs_d)
        nc.sync.dma_start(out=blobw[:, :, W_WV:W_WO], in_=blobw_d[:, :, W_WV:W_WO])
        nc.sync.dma_start(out=blobw[:, :, W_WO:WCOLS], in_=blobw_d[:, :, W_WO:WCOLS])

        sent = blobw[:, :, W_SENT:W_WK]
        wk = blobw[:, :, W_WK:W_WQ]
        wq = blobw[:, :, W_WQ:W_WV]
        wv = blobw[:, :, W_WV:W_WO]
        wo = blobw[:, :, W_WO:WCOLS]
        onesc = blob2[:, B2_ONESC:B2_ONESC + 1]
        ones8 = blobs[:, S_ONES8:S_ONES8 + P]

        def bcast_col(apcol, n):
            return bass.AP(tensor=apcol.tensor, offset=apcol.offset,
                           ap=[apcol.ap[0], [0, n]])

        def emit_qproj_chunk(qt, qin, c):
            pq = ps.tile([P, LBLK], F32, tag="mm")
            for kc in range(NCH):
                mm(pq, wq[:, kc, c * P:(c + 1) * P], qin[:, kc, :],
                   start=(kc == 0), stop=(kc == NCH - 1))
            # bias-add on DVE (0-stride broadcast) keeps ACT free for exps
            nc.vector.tensor_tensor(
                out=qt[:, c, :], in0=pq,
                in1=bcast_col(blobc[:, C_BQ + c:C_BQ + c + 1], LBLK),
                op=OP.add)

        # block-0 q-proj first: wq+qin0 land well before sent+wk
        qt0 = blkp.tile([P, NCH, LBLK], BF16, tag="qt")
        for c in range(NCH):
            emit_qproj_chunk(qt0, qin0, c)

        # ---- k/v projections (once per core) ----
        kt = const.tile([P, NCH, S], BF16, tag="kt")
        vsb = const.tile([P, SC, D], BF16, tag="vsb")
        for c in range(NCH):
            pk = ps.tile([P, S], F32, tag="mm")
            for kc in range(NCH):
                mm(pk, wk[:, kc, c * P:(c + 1) * P], sent[:, kc, :],
                   start=(kc == 0), stop=(kc == NCH - 1))
            nc.scalar.activation(
                out=kt[:, c, :], in_=pk, func=AF.Identity,
                bias=blobc[:, C_BK + c:C_BK + c + 1], scale=1.0)
        for sc in range(SC):
            pv = ps.tile([P, D], F32, tag="mm")
            for kc in range(NCH):
                mm(pv, sent[:, kc, sc * P:(sc + 1) * P], wv[:, kc, :],
                   start=(kc == 0), stop=(kc == NCH - 1))
            nc.vector.tensor_copy(out=vsb[:, sc, :], in_=pv)

        # rstack row 0 carries r (per block); rows 1-7 must be ZERO so the
        # ones8 broadcast matmul never sees 0*garbage.
        rstack = const.tile([H, LBLK], BF16, tag="rstack")
        nc.vector.memset(rstack, 0.0)
        eps_t = const.tile([1, 1], F32, tag="eps")
        nc.vector.memset(eps_t, 1e-5)

        expp = ctx.enter_context(tc.tile_pool(name="expp", bufs=2))
        stat = ctx.enter_context(tc.tile_pool(name="stat", bufs=2))
        scr = ctx.enter_context(tc.tile_pool(name="scr", bufs=2))
        accp = ctx.enter_context(tc.tile_pool(name="acc", bufs=1))

        asb_p = accp.tile([P, NBLK, NCH], F32, tag="asb_p")

        def emit_scores(blk, qt):
            """scores + exp; den mms lag 4 chunks, ctx accumulation (pc)
            groups slot into the exp waits - nothing blocks the in-order
            PE queue on a pending exp."""
            et = expp.tile([P, H * SC, LBLK], BF16, tag="exp")
            ctxu = blkp.tile([P, NCH, LBLK], BF16, tag="ctxu")
            pden = psd.tile([H, LBLK], F32, tag="den")
            NK = H * SC

            def den_mm(k):
                h = k // SC
                mm(pden, blob2[:, B2_SEL8 + h * H:B2_SEL8 + (h + 1) * H],
                   et[:, k, :], start=(k == 0), stop=(k == NK - 1))

            def pc_group(c):
                # unscaled ctx for head pair c -> SBUF early (frees the bank)
                pc = pctx.tile([P, LBLK], F32, tag="pc")
                for sc in range(SC):
                    mm(pc[0:HD, :], vsb[:, sc, c * P:c * P + HD],
                       et[:, (2 * c) * SC + sc, :],
                       start=(sc == 0), stop=(sc == SC - 1))
                for sc in range(SC):
                    mm(pc[HD:P, :], vsb[:, sc, c * P + HD:(c + 1) * P],
                       et[:, (2 * c + 1) * SC + sc, :],
                       start=(sc == 0), stop=(sc == SC - 1))
                nc.vector.tensor_copy(out=ctxu[:, c, :], in_=pc)

            for k in range(NK):
                h, sc = k // SC, k % SC
                pp = (h % 2) * HD
                psc = ps.tile([P, LBLK], F32, tag="mm")
                mm(psc,
                   kt[pp:pp + HD, h // 2, sc * P:(sc + 1) * P],
                   qt[pp:pp + HD, h // 2, :],
                   start=True, stop=True)
                nc.scalar.activation(
                    out=et[:, k, :], in_=psc, func=AF.Exp,
                    bias=blobc[:, C_MASK + sc:C_MASK + sc + 1], scale=1.0)
                if k >= 4:
                    den_mm(k - 4)
                if k in (5, 9, 13):
                    pc_group((k - 5) // 4)
            den_mm(NK - 4)
            pc_group(3)
            for k in range(NK - 3, NK):
                den_mm(k)
            rec = stat.tile([H, LBLK], BF16, tag="rec")
            with nc.allow_low_precision(reason="feeds PE broadcast only"):
                nc.vector.reciprocal(rec, pden)
            return et, ctxu, rec

        def emit_stats_r(blk, pm):
            """r = rsqrt(var+eps) -> rstack row 0.  Mid blocks: Ln+Exp (both
            swaps overlap PE work).  Last block: Sqrt+reciprocal - one swap
            on the serial tail and no ACT op after it."""
            mu2 = stat.tile([1, LBLK], F32, tag="mu2")
            nc.scalar.activation(out=mu2, in_=pm[0:1, :], func=AF.Square,
                                 scale=1.0)
            var = stat.tile([1, LBLK], F32, tag="var")
            nc.vector.tensor_tensor(out=var, in0=pm[32:33, :], in1=mu2,
                                    op=OP.subtract)
            lnv = stat.tile([1, LBLK], F32, tag="lnv")
            if blk == NBLK - 1:
                nc.scalar.activation(out=lnv, in_=var, func=AF.Sqrt,
                                     bias=eps_t, scale=1.0)
                with nc.allow_low_precision(reason="r weight, bf16 ok"):
                    nc.vector.reciprocal(rstack[0:1, :], lnv)
            else:
                nc.scalar.activation(out=lnv, in_=var, func=AF.Ln,
                                     bias=eps_t, scale=1.0)
                with nc.allow_low_precision(reason="r weight, bf16 ok"):
                    nc.scalar.activation(out=rstack[0:1, :], in_=lnv,
                                         func=AF.Exp, scale=-0.5)

        def emit_stats_s2(blk, xt):
            """asb_p[:, blk, :] = sum_l xt*r (r broadcast via ones8 matmul)."""
            prs_p = psd.tile([H, LBLK], F32, tag="den")
            prs_pf = bass.AP(tensor=prs_p.tensor, offset=prs_p.offset,
                             ap=[[prs_p.ap[0][0], P], prs_p.ap[1]])
            mm(prs_pf, ones8, rstack, start=True, stop=True)
            prs = scr.tile([P, LBLK], BF16, tag="prs")
            nc.vector.tensor_copy(out=prs, in_=prs_pf)
            prs_b = bass.AP(tensor=prs.tensor, offset=prs.offset,
                            ap=[prs.ap[0], [0, NCH], prs.ap[1]])
            s2 = scr.tile([P, NCH, LBLK], BF16, tag="s2")
            with nc.allow_low_precision(reason="weighted partial sums"):
                nc.vector.tensor_tensor(out=s2, in0=xt, in1=prs_b, op=OP.mult)
                # halving tree: tensor_tensor has a 2x bf16 mode,
                # tensor_reduce does not
                t1 = scr.tile([P, NCH, LBLK // 2], BF16, tag="t1")
                nc.vector.tensor_tensor(
                    out=t1, in0=s2[:, :, 0:LBLK // 2],
                    in1=s2[:, :, LBLK // 2:LBLK], op=OP.add)
                t2 = scr.tile([P, NCH, LBLK // 4], BF16, tag="t2")
                nc.vector.tensor_tensor(
                    out=t2, in0=t1[:, :, 0:LBLK // 4],
                    in1=t1[:, :, LBLK // 4:LBLK // 2], op=OP.add)
            nc.vector.tensor_reduce(
                out=asb_p[:, blk, :], in_=t2,
                axis=mybir.AxisListType.X, op=OP.add)

        def emit_back(blk, qin, ctxu, rec):
            """scale ctx by 1/den, out-projection + residual, mu/sq sums."""
            ctxs = blkp.tile([P, NCH, LBLK], BF16, tag="ctxs")
            for c in range(NCH):
                prb = ps.tile([P, LBLK], F32, tag="mm")
                mm(prb, blobs[:, S_SEL2 + c * P:S_SEL2 + (c + 1) * P], rec,
                   start=True, stop=True)
                nc.vector.tensor_tensor(
                    out=ctxs[:, c, :], in0=ctxu[:, c, :], in1=prb, op=OP.mult)
            xt = blkp.tile([P, NCH, LBLK], BF16, tag="xt")
            pm = psm.tile([33, LBLK], F32, tag="pm")
            for e in range(NCH):
                po = ps.tile([P, LBLK], F32, tag="mm")
                for kc in range(NCH):
                    mm(po, wo[:, kc, e * P:(e + 1) * P], ctxs[:, kc, :],
                       start=(kc == 0), stop=(kc == NCH - 1))
                po2 = scr.tile([P, LBLK], BF16, tag="po2")
                nc.scalar.activation(
                    out=po2, in_=po, func=AF.Identity,
                    bias=blobc[:, C_BO + e:C_BO + e + 1], scale=1.0)
                nc.vector.tensor_tensor(
                    out=xt[:, e, :], in0=qin[:, e, :], in1=po2, op=OP.add)
                mm(pm[0:1, :], onesc, xt[:, e, :],
                   start=(e == 0), stop=(e == NCH - 1))
                x2 = scr.tile([P, LBLK], BF16, tag="x2")
                nc.vector.tensor_tensor(
                    out=x2, in0=xt[:, e, :], in1=xt[:, e, :], op=OP.mult)
                mm(pm[32:33, :], onesc, x2,
                   start=(e == 0), stop=(e == NCH - 1))
            return xt, pm

        # ---- main loop; stats pipelined one block behind ----
        qin_cur, qt_cur = qin0, qt0
        carry = None
        for blk in range(NBLK):
            if blk + 1 < NBLK:
                qin_nxt = blkp.tile([P, NCH, LBLK], BF16, tag="qin")
                nc.sync.dma_start(
                    out=qin_nxt,
                    in_=qin_d[:, :, (blk + 1) * LBLK:(blk + 2) * LBLK])
            else:
                qin_nxt = None
            et, ctxu, rec = emit_scores(blk, qt_cur)
            qt_nxt = None
            if qin_nxt is not None:
                qt_nxt = blkp.tile([P, NCH, LBLK], BF16, tag="qt")
                for c in range(2):
                    emit_qproj_chunk(qt_nxt, qin_nxt, c)
            if carry is not None:
                emit_stats_r(blk - 1, carry[1])
            xt, pm = emit_back(blk, qin_cur, ctxu, rec)
            if qt_nxt is not None:
                for c in range(2, NCH):
                    emit_qproj_chunk(qt_nxt, qin_nxt, c)
            if carry is not None:
                emit_stats_s2(blk - 1, carry[0])
            carry = (xt, pm)
            if qin_nxt is not None:
                qin_cur, qt_cur = qin_nxt, qt_nxt
        emit_stats_r(NBLK - 1, carry[1])
        emit_stats_s2(NBLK - 1, carry[0])

        # final combine happens on the host (516 floats)
        nc.sync.dma_start(out=asbp_d, in_=asb_p)

    nc.compile()
    return nc


def make_inputs(top_word_vecs, sent_vecs, num_sents, Wq, bq, Wk, bk, Wv, bv,
                Wo, bo, ln_w, ln_b, np_dt=None):
    """Host-side prep: transposes/layouts + per-core sharding over batch."""
    if np_dt is None:
        import ml_dtypes
        np_dt = ml_dtypes.bfloat16
    f32 = np.float32
    scale = 1.0 / np.sqrt(HD)

    def wcol(W):  # [dout, din] -> lhsT layout [128, 4 din-chunks, 512 dout]
        wt = np.ascontiguousarray(np.asarray(W, f32).T)
        return wt.reshape(NCH, P, D).transpose(1, 0, 2)

    def col(v):  # [512] -> [128, 4]
        return np.ascontiguousarray(np.asarray(v, f32).reshape(NCH, P).T)

    sv = np.asarray(sent_vecs, f32)
    twv = np.asarray(top_word_vecs, f32).reshape(B, L, D)
    ns = np.asarray(num_sents).astype(np.int64)

    blob2 = np.zeros((P, B2COLS), f32)
    for h in range(H):
        blob2[:, B2_SEL8 + h * H + h] = 1.0
    blob2[:, B2_ONESC] = 1.0 / D

    blobs = np.zeros((H, SCOLS), f32)
    for c in range(NCH):
        blobs[2 * c, S_SEL2 + c * P:S_SEL2 + c * P + HD] = 1.0
        blobs[2 * c + 1, S_SEL2 + c * P + HD:S_SEL2 + (c + 1) * P] = 1.0
    blobs[0, S_ONES8:S_ONES8 + P] = 1.0

    cshared = np.zeros((P, CCOLS), f32)
    cshared[:, C_BQ:C_BQ + NCH] = col(np.asarray(bq, f32) * scale)
    cshared[:, C_BK:C_BK + NCH] = col(bk)
    cshared[:, C_BO:C_BO + NCH] = col(
        np.asarray(bo, f32) + np.asarray(Wo, f32) @ np.asarray(bv, f32))
    cshared[:, C_LNW:C_LNW + NCH] = col(np.asarray(ln_w, f32) / L)
    cshared[:, C_LNB:C_LNB + NCH] = col(ln_b)

    wq_l = wcol(np.asarray(Wq, f32) * scale)
    wk_l = wcol(Wk)
    wv_l = wcol(Wv)
    wo_l = wcol(Wo)

    sidx = np.arange(S).reshape(SC, P).T  # [p, sc] -> s
    in_maps = []
    for b in range(B):
        blobw = np.empty((P, NCH, WCOLS), f32)
        st = np.ascontiguousarray(sv[b].T).reshape(NCH, P, S).transpose(1, 0, 2)
        blobw[:, :, W_SENT:W_WK] = st
        blobw[:, :, W_WK:W_WQ] = wk_l
        blobw[:, :, W_WQ:W_WV] = wq_l
        blobw[:, :, W_WV:W_WO] = wv_l
        blobw[:, :, W_WO:WCOLS] = wo_l
        qin_t = np.ascontiguousarray(twv[b].T).reshape(NCH, P, L)
        qin_t = qin_t.transpose(1, 0, 2)
        blobc = cshared.copy()
        mask = np.zeros((P, SC), f32)
        mask[sidx >= ns[b]] = NEG
        blobc[:, C_MASK:C_MASK + SC] = mask
        in_maps.append({
            "qin_t": np.ascontiguousarray(qin_t).astype(np_dt),
            "blobw": np.ascontiguousarray(blobw).astype(np_dt),
            "blob2": blob2.astype(np_dt),
            "blobs": blobs.astype(np_dt),
            "blobc": blobc,
        })
    return in_maps


_NC_CACHE = {}


def _get_nc():
    key = "v2"
    if key not in _NC_CACHE:
        _NC_CACHE[key] = build_kernel()
    return _NC_CACHE[key]


def _host_combine(asbp, ln_w, ln_b):
    """Final LayerNorm combine on 516 floats per batch element:
    out[d] = ln_w[d]/L * (asb[d] - bscal) + ln_b[d], bscal = sum_d asb / D."""
    asb = np.asarray(asbp, np.float32).sum(axis=1)      # [P, NCH]
    bscal = asb.sum() / D
    lnw = np.asarray(ln_w, np.float32).reshape(NCH, P).T / L
    lnb = np.asarray(ln_b, np.float32).reshape(NCH, P).T
    t = (asb - bscal) * lnw + lnb                        # [P, NCH]
    return np.ascontiguousarray(t.T).reshape(D)          # d = c*P + p


def kernel(**inputs):
    nc = _get_nc()
    in_maps = make_inputs(**inputs)
    res = run_bass_kernel_spmd(nc, in_maps, list(range(B)))
    out = np.stack([
        _host_combine(res.results[i]["asbp"], inputs["ln_w"], inputs["ln_b"])
        for i in range(B)]).astype(np.float32)
    return out


def _make_sharded(nc, in_maps):
    """Replicate bass2jax.run_bass_via_pjrt's jit/shard_map wiring but
    return a callable over pre-placed device arrays for repeat timing."""
    import jax
    import concourse.mybir as mb
    from concourse import bass2jax
    from jax.sharding import Mesh, PartitionSpec, NamedSharding
    from jax.experimental.shard_map import shard_map

    bass2jax.install_neuronx_cc_hook()
    pid_name = nc.partition_id_tensor.name if nc.partition_id_tensor else None
    in_names, out_names, out_avals = [], [], []
    for alloc in nc.m.functions[0].allocations:
        if not isinstance(alloc, mb.MemoryLocationSet):
            continue
        name = alloc.memorylocations[0].name
        if alloc.kind == "ExternalInput":
            if name != pid_name:
                in_names.append(name)
        elif alloc.kind == "ExternalOutput":
            out_names.append(name)
            out_avals.append(
                jax.core.ShapedArray(tuple(alloc.tensor_shape),
                                     mb.dt.np(alloc.dtype)))
    n_params = len(in_names)
    all_names = in_names + out_names

    def _body(*args):
        operands = list(args)
        if pid_name is not None:
            operands.append(bass2jax.partition_id_tensor())
        outs = bass2jax._bass_exec_p.bind(
            *operands,
            out_avals=tuple(out_avals),
            in_names=tuple(all_names + ([pid_name] if pid_name else [])),
            out_names=tuple(out_names),
            lowering_input_output_aliases=(),
            sim_require_finite=True,
            sim_require_nnan=True,
            nc=nc,
        )
        return tuple(outs)

    devices = jax.devices()[:B]
    mesh = Mesh(np.asarray(devices), ("core",))
    spec = PartitionSpec("core")
    nouts = len(out_names)
    sharded = jax.jit(
        shard_map(_body, mesh=mesh, in_specs=(spec,) * (n_params + nouts),
                  out_specs=(spec,) * nouts, check_rep=False),
        keep_unused=True)
    sh = NamedSharding(mesh, spec)
    args = []
    for i, name in enumerate(in_names):
        cat = np.concatenate([np.asarray(m[name]) for m in in_maps], axis=0)
        args.append(jax.device_put(cat, sh))
    for av in out_avals:
        z = np.zeros((B * av.shape[0], *av.shape[1:]), av.dtype)
        args.append(jax.device_put(z, sh))
    return sharded, args, out_names, out_avals


def bench(n_iters=30, **inputs):
    import jax
    nc = _get_nc()
    in_maps = make_inputs(**inputs)
    sharded, args, out_names, out_avals = _make_sharded(nc, in_maps)
    out = sharded(*args)
    jax.block_until_ready(out)
    t0 = time.perf_counter()
    for _ in range(n_iters):
        out = sharded(*args)
    jax.block_until_ready(out)
    t1 = time.perf_counter()
    return (t1 - t0) / n_iters * 1e9






# revision 4
# speedup vs baseline: 1.5798x; 1.0375x over previous
"""DocumentCrossAttentionMHA Trainium2 kernel, v2.

Data-parallel over batch: each of the 8 NeuronCores computes one batch
element end-to-end.  Same transposed dataflow as v1 (every matmul
consumes its producer's layout directly), plus:

  - bscal (= sum_l mu_l r_l) is derived from asb (sum_d asb[d] = D*bscal),
    so the epilogue needs no mu*r reduction and no DRAM round-trips;
    partition broadcasts ride tiny PE matmuls against an 8-row ones
    selector (rstack trick) instead of DRAM round-trip DMAs.
  - rsqrt for LayerNorm as r = exp(-0.5*ln(var+eps)) (Ln/Exp/Identity/
    Square share one activation-table set with the softmax Exp).  The
    stats stage is software-pipelined one block behind the main loop so
    the remaining table swaps overlap PE-heavy phases instead of
    stalling the softmax exp stream; the last block uses Sqrt + DVE
    reciprocal (one swap on the serial tail, no ACT op after it).
  - Everything ordered for the in-order per-engine queues: den
    column-sum matmuls lag their exp producers by 4 chunks, ctx
    accumulation slots into the exp-wait windows (unscaled ctx is
    copied out of PSUM early to free banks), next-block q-proj chunks
    fill the den-tail / reciprocal latency.
  - Block-0 q-projection is emitted before the k/v projections: engine
    queues issue in order, and wq+qin0 arrive from HBM well before
    sent+wk, so the PE starts ~3us earlier.
  - s2 reduction as a bf16 halving tree (tensor_reduce has no 2x DVE
    mode; tensor_tensor does).
  - mu/sq share one PSUM bank (rows 0/32); ctx head pairs share one
    bank ([0:64)+[64:128)) via partition-offset matmul outputs
    (matmul output base partition must be 0, 32 or 64).
  - Inputs packed into 5 tensors (was 18) to cut dispatch/DMA overhead;
    the final LayerNorm combine (516 floats/core) happens on the host.

Hardware landmines (crash NRT or fail the BIR verifier):
  - no single-partition-contraction (K=1) fp32 matmuls
  - no tensor_tensor_reduce custom DVE op
  - no float32r operands
  - DVE tensor_tensor may read at most ONE operand from PSUM
"""

import time
from contextlib import ExitStack

import numpy as np

import concourse.bacc as bacc
import concourse.bass as bass
import concourse.mybir as mybir
import concourse.tile as tile
from concourse.bass_utils import run_bass_kernel_spmd

B, S, KTOP, D, H = 8, 256, 8, 512, 8
HD = D // H          # 64
L = S * KTOP         # 2048
P = 128
NCH = D // P         # 4 feature chunks
SC = S // P          # 2 s chunks
LBLK = 512
NBLK = L // LBLK     # 4 l blocks
NEG = -1.0e9
F32 = mybir.dt.float32
BF16 = mybir.dt.bfloat16

# blobW column offsets (bf16 [P, NCH, WCOLS]): sent | wk | wq | wv | wo
W_SENT, W_WK, W_WQ, W_WV, W_WO = 0, S, S + D, S + 2 * D, S + 3 * D
WCOLS = S + 4 * D    # 2304
# blobC column offsets (f32 [P, CCOLS]): bq | bk | bo' | mask | lnw | lnb
C_BQ, C_BK, C_BO, C_MASK = 0, NCH, 2 * NCH, 3 * NCH
C_LNW, C_LNB = 3 * NCH + SC, 4 * NCH + SC
CCOLS = 5 * NCH + SC  # 22
# blobS (bf16 [8, SCOLS]): sel2 flat | onesrow8
S_SEL2, S_ONES8 = 0, NCH * P
SCOLS = NCH * P + P  # 640
# blob2 (bf16 [P, 65]): sel8 flat | onesc (1/D)
B2_SEL8, B2_ONESC = 0, H * H
B2COLS = H * H + 1


def build_kernel():
    nc = bacc.Bacc(trn_type="TRN2", debug=False)
    AF = mybir.ActivationFunctionType
    OP = mybir.AluOpType

    def mm(out, lhsT, rhs, **kw):
        nc.tensor.matmul(out, lhsT, rhs, **kw)

    qin_d = nc.dram_tensor("qin_t", [P, NCH, L], BF16, kind="ExternalInput").ap()
    blobw_d = nc.dram_tensor("blobw", [P, NCH, WCOLS], BF16, kind="ExternalInput").ap()
    blob2_d = nc.dram_tensor("blob2", [P, B2COLS], BF16, kind="ExternalInput").ap()
    blobs_d = nc.dram_tensor("blobs", [H, SCOLS], BF16, kind="ExternalInput").ap()
    blobc_d = nc.dram_tensor("blobc", [P, CCOLS], F32, kind="ExternalInput").ap()
    asbp_d = nc.dram_tensor("asbp", [P, NBLK, NCH], F32, kind="ExternalOutput").ap()

    with tile.TileContext(nc) as tc, ExitStack() as ctx:
        const = ctx.enter_context(tc.tile_pool(name="const", bufs=1))
        blkp = ctx.enter_context(tc.tile_pool(name="blk", bufs=3))
        ps = ctx.enter_context(tc.tile_pool(name="ps", bufs=4, space="PSUM"))
        pctx = ctx.enter_context(tc.tile_pool(name="pctx", bufs=2, space="PSUM"))
        psd = ctx.enter_context(tc.tile_pool(name="psd", bufs=1, space="PSUM"))
        psm = ctx.enter_context(tc.tile_pool(name="psm", bufs=1, space="PSUM"))

        # ---- constant loads, ordered by first consumption ----
        blobw = const.tile([P, NCH, WCOLS], BF16, tag="blobw")
        qin0 = blkp.tile([P, NCH, LBLK], BF16, tag="qin")
        nc.sync.dma_start(out=blobw[:, :, W_WQ:W_WV], in_=blobw_d[:, :, W_WQ:W_WV])
        nc.sync.dma_start(out=qin0, in_=qin_d[:, :, 0:LBLK])
        blobc = const.tile([P, CCOLS], F32, tag="blobc")
        nc.sync.dma_start(out=blobc, in_=blobc_d)
        nc.sync.dma_start(out=blobw[:, :, 0:W_WQ], in_=blobw_d[:, :, 0:W_WQ])
        blob2 = const.tile([P, B2COLS], BF16, tag="blob2")
        nc.sync.dma_start(out=blob2, in_=blob2_d)
        blobs = const.tile([H, SCOLS], BF16, tag="blobs")
        nc.sync.dma_start(out=blobs, in_=blobs_d)
        nc.sync.dma_start(out=blobw[:, :, W_WV:W_WO], in_=blobw_d[:, :, W_WV:W_WO])
        nc.sync.dma_start(out=blobw[:, :, W_WO:WCOLS], in_=blobw_d[:, :, W_WO:WCOLS])

        sent = blobw[:, :, W_SENT:W_WK]
        wk = blobw[:, :, W_WK:W_WQ]
        wq = blobw[:, :, W_WQ:W_WV]
        wv = blobw[:, :, W_WV:W_WO]
        wo = blobw[:, :, W_WO:WCOLS]
        onesc = blob2[:, B2_ONESC:B2_ONESC + 1]
        ones8 = blobs[:, S_ONES8:S_ONES8 + P]

        def bcast_col(apcol, n):
            return bass.AP(tensor=apcol.tensor, offset=apcol.offset,
                           ap=[apcol.ap[0], [0, n]])

        def emit_qproj_chunk(qt, qin, c):
            pq = ps.tile([P, LBLK], F32, tag="mm")
            for kc in range(NCH):
                mm(pq, wq[:, kc, c * P:(c + 1) * P], qin[:, kc, :],
                   start=(kc == 0), stop=(kc == NCH - 1))
            # bias-add on DVE (0-stride broadcast) keeps ACT free for exps
            nc.vector.tensor_tensor(
                out=qt[:, c, :], in0=pq,
                in1=bcast_col(blobc[:, C_BQ + c:C_BQ + c + 1], LBLK),
                op=OP.add)

        # block-0 q-proj first: wq+qin0 land well before sent+wk
        qt0 = blkp.tile([P, NCH, LBLK], BF16, tag="qt")
        for c in range(NCH):
            emit_qproj_chunk(qt0, qin0, c)

        # ---- k/v projections (once per core) ----
        kt = const.tile([P, NCH, S], BF16, tag="kt")
        vsb = const.tile([P, SC, D], BF16, tag="vsb")
        for c in range(NCH):
            pk = ps.tile([P, S], F32, tag="mm")
            for kc in range(NCH):
                mm(pk, wk[:, kc, c * P:(c + 1) * P], sent[:, kc, :],
                   start=(kc == 0), stop=(kc == NCH - 1))
            nc.scalar.activation(
                out=kt[:, c, :], in_=pk, func=AF.Identity,
                bias=blobc[:, C_BK + c:C_BK + c + 1], scale=1.0)
        for sc in range(SC):
            pv = ps.tile([P, D], F32, tag="mm")
            for kc in range(NCH):
                mm(pv, sent[:, kc, sc * P:(sc + 1) * P], wv[:, kc, :],
                   start=(kc == 0), stop=(kc == NCH - 1))
            nc.vector.tensor_copy(out=vsb[:, sc, :], in_=pv)

        # rstack row 0 carries r (per block); rows 1-7 must be ZERO so the
        # ones8 broadcast matmul never sees 0*garbage.
        rstack = const.tile([H, LBLK], BF16, tag="rstack")
        nc.vector.memset(rstack, 0.0)
        eps_t = const.tile([1, 1], F32, tag="eps")
        nc.vector.memset(eps_t, 1e-5)

        expp = ctx.enter_context(tc.tile_pool(name="expp", bufs=2))
        stat = ctx.enter_context(tc.tile_pool(name="stat", bufs=2))
        scr = ctx.enter_context(tc.tile_pool(name="scr", bufs=2))
        accp = ctx.enter_context(tc.tile_pool(name="acc", bufs=1))

        asb_p = accp.tile([P, NBLK, NCH], F32, tag="asb_p")

        def emit_scores(blk, qt):
            """scores + exp; den mms lag 4 chunks, ctx accumulation (pc)
            groups slot into the exp waits - nothing blocks the in-order
            PE queue on a pending exp."""
            et = expp.tile([P, H * SC, LBLK], BF16, tag="exp")
            ctxu = blkp.tile([P, NCH, LBLK], BF16, tag="ctxu")
            pden = psd.tile([H, LBLK], F32, tag="den")
            NK = H * SC

            def den_mm(k):
                h = k // SC
                mm(pden, blob2[:, B2_SEL8 + h * H:B2_SEL8 + (h + 1) * H],
                   et[:, k, :], start=(k == 0), stop=(k == NK - 1))

            def pc_group(c):
                # unscaled ctx for head pair c -> SBUF early (frees the bank)
                pc = pctx.tile([P, LBLK], F32, tag="pc")
                for sc in range(SC):
                    mm(pc[0:HD, :], vsb[:, sc, c * P:c * P + HD],
                       et[:, (2 * c) * SC + sc, :],
                       start=(sc == 0), stop=(sc == SC - 1))
                for sc in range(SC):
                    mm(pc[HD:P, :], vsb[:, sc, c * P + HD:(c + 1) * P],
                       et[:, (2 * c + 1) * SC + sc, :],
                       start=(sc == 0), stop=(sc == SC - 1))
                nc.vector.tensor_copy(out=ctxu[:, c, :], in_=pc)

            for k in range(NK):
                h, sc = k // SC, k % SC
                pp = (h % 2) * HD
                psc = ps.tile([P, LBLK], F32, tag="mm")
                mm(psc,
                   kt[pp:pp + HD, h // 2, sc * P:(sc + 1) * P],
                   qt[pp:pp + HD, h // 2, :],
                   start=True, stop=True)
                nc.scalar.activation(
                    out=et[:, k, :], in_=psc, func=AF.Exp,
                    bias=blobc[:, C_MASK + sc:C_MASK + sc + 1], scale=1.0)
                if k >= 4:
                    den_mm(k - 4)
                if k in (5, 9, 13):
                    pc_group((k - 5) // 4)
            den_mm(NK - 4)
            pc_group(3)
            for k in range(NK - 3, NK):
                den_mm(k)
            rec = stat.tile([H, LBLK], BF16, tag="rec")
            with nc.allow_low_precision(reason="feeds PE broadcast only"):
                nc.vector.reciprocal(rec, pden)
            return et, ctxu, rec

        def emit_stats_r(blk, pm):
            """r = rsqrt(var+eps) -> rstack row 0.  Mid blocks: Ln+Exp (both
            swaps overlap PE work).  Last block: Sqrt+reciprocal - one swap
            on the serial tail and no ACT op after it."""
            mu2 = stat.tile([1, LBLK], F32, tag="mu2")
            nc.scalar.activation(out=mu2, in_=pm[0:1, :], func=AF.Square,
                                 scale=1.0)
            var = stat.tile([1, LBLK], F32, tag="var")
            nc.vector.tensor_tensor(out=var, in0=pm[32:33, :], in1=mu2,
                                    op=OP.subtract)
            lnv = stat.tile([1, LBLK], F32, tag="lnv")
            if blk == NBLK - 1:
                nc.scalar.activation(out=lnv, in_=var, func=AF.Sqrt,
                                     bias=eps_t, scale=1.0)
                with nc.allow_low_precision(reason="r weight, bf16 ok"):
                    nc.vector.reciprocal(rstack[0:1, :], lnv)
            else:
                nc.scalar.activation(out=lnv, in_=var, func=AF.Ln,
                                     bias=eps_t, scale=1.0)
                with nc.allow_low_precision(reason="r weight, bf16 ok"):
                    nc.scalar.activation(out=rstack[0:1, :], in_=lnv,
                                         func=AF.Exp, scale=-0.5)

        def emit_stats_s2(blk, xt):
            """asb_p[:, blk, :] = sum_l xt*r (r broadcast via ones8 matmul)."""
            prs_p = psd.tile([H, LBLK], F32, tag="den")
            prs_pf = bass.AP(tensor=prs_p.tensor, offset=prs_p.offset,
                             ap=[[prs_p.ap[0][0], P], prs_p.ap[1]])
            mm(prs_pf, ones8, rstack, start=True, stop=True)
            prs = scr.tile([P, LBLK], BF16, tag="prs")
            nc.vector.tensor_copy(out=prs, in_=prs_pf)
            prs_b = bass.AP(tensor=prs.tensor, offset=prs.offset,
                            ap=[prs.ap[0], [0, NCH], prs.ap[1]])
            s2 = scr.tile([P, NCH, LBLK], BF16, tag="s2")
            with nc.allow_low_precision(reason="weighted partial sums"):
                nc.vector.tensor_tensor(out=s2, in0=xt, in1=prs_b, op=OP.mult)
                # halving tree: tensor_tensor has a 2x bf16 mode,
                # tensor_reduce does not
                t1 = scr.tile([P, NCH, LBLK // 2], BF16, tag="t1")
                nc.vector.tensor_tensor(
                    out=t1, in0=s2[:, :, 0:LBLK // 2],
                    in1=s2[:, :, LBLK // 2:LBLK], op=OP.add)
                t2 = scr.tile([P, NCH, LBLK // 4], BF16, tag="t2")
                nc.vector.tensor_tensor(
                    out=t2, in0=t1[:, :, 0:LBLK // 4],
                    in1=t1[:, :, LBLK // 4:LBLK // 2], op=OP.add)
            nc.vector.tensor_reduce(
                out=asb_p[:, blk, :], in_=t2,
                axis=mybir.AxisListType.X, op=OP.add)

        def emit_back(blk, qin, ctxu, rec):
            """scale ctx by 1/den, out-projection + residual, mu/sq sums."""
            ctxs = blkp.tile([P, NCH, LBLK], BF16, tag="ctxs")
            for c in range(NCH):
                prb = ps.tile([P, LBLK], F32, tag="mm")
                mm(prb, blobs[:, S_SEL2 + c * P:S_SEL2 + (c + 1) * P], rec,
                   start=True, stop=True)
                nc.vector.tensor_tensor(
                    out=ctxs[:, c, :], in0=ctxu[:, c, :], in1=prb, op=OP.mult)
            xt = blkp.tile([P, NCH, LBLK], BF16, tag="xt")
            pm = psm.tile([33, LBLK], F32, tag="pm")
            for e in range(NCH):
                po = ps.tile([P, LBLK], F32, tag="mm")
                for kc in range(NCH):
                    mm(po, wo[:, kc, e * P:(e + 1) * P], ctxs[:, kc, :],
                       start=(kc == 0), stop=(kc == NCH - 1))
                po2 = scr.tile([P, LBLK], BF16, tag="po2")
                nc.scalar.activation(
                    out=po2, in_=po, func=AF.Identity,
                    bias=blobc[:, C_BO + e:C_BO + e + 1], scale=1.0)
                nc.vector.tensor_tensor(
                    out=xt[:, e, :], in0=qin[:, e, :], in1=po2, op=OP.add)
                mm(pm[0:1, :], onesc, xt[:, e, :],
                   start=(e == 0), stop=(e == NCH - 1))
                x2 = scr.tile([P, LBLK], BF16, tag="x2")
                nc.vector.tensor_tensor(
                    out=x2, in0=xt[:, e, :], in1=xt[:, e, :], op=OP.mult)
                mm(pm[32:33, :], onesc, x2,
                   start=(e == 0), stop=(e == NCH - 1))
            return xt, pm

        # ---- main loop; stats pipelined one block behind ----
        qin_cur, qt_cur = qin0, qt0
        carry = None
        for blk in range(NBLK):
            if blk + 1 < NBLK:
                qin_nxt = blkp.tile([P, NCH, LBLK], BF16, tag="qin")
                nc.sync.dma_start(
                    out=qin_nxt,
                    in_=qin_d[:, :, (blk + 1) * LBLK:(blk + 2) * LBLK])
            else:
                qin_nxt = None
            et, ctxu, rec = emit_scores(blk, qt_cur)
            qt_nxt = None
            if qin_nxt is not None:
                qt_nxt = blkp.tile([P, NCH, LBLK], BF16, tag="qt")
                for c in range(2):
                    emit_qproj_chunk(qt_nxt, qin_nxt, c)
            if carry is not None:
                emit_stats_r(blk - 1, carry[1])
            xt, pm = emit_back(blk, qin_cur, ctxu, rec)
            if qt_nxt is not None:
                for c in range(2, NCH):
                    emit_qproj_chunk(qt_nxt, qin_nxt, c)
            if carry is not None:
                emit_stats_s2(blk - 1, carry[0])
            carry = (xt, pm)
            if qin_nxt is not None:
                qin_cur, qt_cur = qin_nxt, qt_nxt
        emit_stats_r(NBLK - 1, carry[1])
        emit_stats_s2(NBLK - 1, carry[0])

        # final combine happens on the host (516 floats)
        nc.sync.dma_start(out=asbp_d, in_=asb_p)

    nc.compile()
    return nc


def make_inputs(top_word_vecs, sent_vecs, num_sents, Wq, bq, Wk, bk, Wv, bv,
                Wo, bo, ln_w, ln_b, np_dt=None):
    """Host-side prep: transposes/layouts + per-core sharding over batch."""
    if np_dt is None:
        import ml_dtypes
        np_dt = ml_dtypes.bfloat16
    f32 = np.float32
    scale = 1.0 / np.sqrt(HD)

    def wcol(W):  # [dout, din] -> lhsT layout [128, 4 din-chunks, 512 dout]
        wt = np.ascontiguousarray(np.asarray(W, f32).T)
        return wt.reshape(NCH, P, D).transpose(1, 0, 2)

    def col(v):  # [512] -> [128, 4]
        return np.ascontiguousarray(np.asarray(v, f32).reshape(NCH, P).T)

    sv = np.asarray(sent_vecs, f32)
    twv = np.asarray(top_word_vecs, f32).reshape(B, L, D)
    ns = np.asarray(num_sents).astype(np.int64)

    blob2 = np.zeros((P, B2COLS), f32)
    for h in range(H):
        blob2[:, B2_SEL8 + h * H + h] = 1.0
    blob2[:, B2_ONESC] = 1.0 / D

    blobs = np.zeros((H, SCOLS), f32)
    for c in range(NCH):
        blobs[2 * c, S_SEL2 + c * P:S_SEL2 + c * P + HD] = 1.0
        blobs[2 * c + 1, S_SEL2 + c * P + HD:S_SEL2 + (c + 1) * P] = 1.0
    blobs[0, S_ONES8:S_ONES8 + P] = 1.0

    cshared = np.zeros((P, CCOLS), f32)
    cshared[:, C_BQ:C_BQ + NCH] = col(np.asarray(bq, f32) * scale)
    cshared[:, C_BK:C_BK + NCH] = col(bk)
    cshared[:, C_BO:C_BO + NCH] = col(
        np.asarray(bo, f32) + np.asarray(Wo, f32) @ np.asarray(bv, f32))
    cshared[:, C_LNW:C_LNW + NCH] = col(np.asarray(ln_w, f32) / L)
    cshared[:, C_LNB:C_LNB + NCH] = col(ln_b)

    wq_l = wcol(np.asarray(Wq, f32) * scale)
    wk_l = wcol(Wk)
    wv_l = wcol(Wv)
    wo_l = wcol(Wo)

    sidx = np.arange(S).reshape(SC, P).T  # [p, sc] -> s
    in_maps = []
    for b in range(B):
        blobw = np.empty((P, NCH, WCOLS), f32)
        st = np.ascontiguousarray(sv[b].T).reshape(NCH, P, S).transpose(1, 0, 2)
        blobw[:, :, W_SENT:W_WK] = st
        blobw[:, :, W_WK:W_WQ] = wk_l
        blobw[:, :, W_WQ:W_WV] = wq_l
        blobw[:, :, W_WV:W_WO] = wv_l
        blobw[:, :, W_WO:WCOLS] = wo_l
        qin_t = np.ascontiguousarray(twv[b].T).reshape(NCH, P, L)
        qin_t = qin_t.transpose(1, 0, 2)
        blobc = cshared.copy()
        mask = np.zeros((P, SC), f32)
        mask[sidx >= ns[b]] = NEG
        blobc[:, C_MASK:C_MASK + SC] = mask
        in_maps.append({
            "qin_t": np.ascontiguousarray(qin_t).astype(np_dt),
            "blobw": np.ascontiguousarray(blobw).astype(np_dt),
            "blob2": blob2.astype(np_dt),
            "blobs": blobs.astype(np_dt),
            "blobc": blobc,
        })
    return in_maps


_NC_CACHE = {}


def _get_nc():
    key = "v2"
    if key not in _NC_CACHE:
        _NC_CACHE[key] = build_kernel()
    return _NC_CACHE[key]


def _host_combine(asbp, ln_w, ln_b):
    """Final LayerNorm combine on 516 floats per batch element:
    out[d] = ln_w[d]/L * (asb[d] - bscal) + ln_b[d], bscal = sum_d asb / D."""
    asb = np.asarray(asbp, np.float32).sum(axis=1)      # [P, NCH]
    bscal = asb.sum() / D
    lnw = np.asarray(ln_w, np.float32).reshape(NCH, P).T / L
    lnb = np.asarray(ln_b, np.float32).reshape(NCH, P).T
    t = (asb - bscal) * lnw + lnb                        # [P, NCH]
    return np.ascontiguousarray(t.T).reshape(D)          # d = c*P + p


def kernel(**inputs):
    nc = _get_nc()
    in_maps = make_inputs(**inputs)
    res = run_bass_kernel_spmd(nc, in_maps, list(range(B)))
    out = np.stack([
        _host_combine(res.results[i]["asbp"], inputs["ln_w"], inputs["ln_b"])
        for i in range(B)]).astype(np.float32)
    return out


def _make_sharded(nc, in_maps):
    """Replicate bass2jax.run_bass_via_pjrt's jit/shard_map wiring but
    return a callable over pre-placed device arrays for repeat timing."""
    import jax
    import concourse.mybir as mb
    from concourse import bass2jax
    from jax.sharding import Mesh, PartitionSpec, NamedSharding
    from jax.experimental.shard_map import shard_map

    bass2jax.install_neuronx_cc_hook()
    pid_name = nc.partition_id_tensor.name if nc.partition_id_tensor else None
    in_names, out_names, out_avals = [], [], []
    for alloc in nc.m.functions[0].allocations:
        if not isinstance(alloc, mb.MemoryLocationSet):
            continue
        name = alloc.memorylocations[0].name
        if alloc.kind == "ExternalInput":
            if name != pid_name:
                in_names.append(name)
        elif alloc.kind == "ExternalOutput":
            out_names.append(name)
            out_avals.append(
                jax.core.ShapedArray(tuple(alloc.tensor_shape),
                                     mb.dt.np(alloc.dtype)))
    n_params = len(in_names)
    all_names = in_names + out_names

    def _body(*args):
        operands = list(args)
        if pid_name is not None:
            operands.append(bass2jax.partition_id_tensor())
        outs = bass2jax._bass_exec_p.bind(
            *operands,
            out_avals=tuple(out_avals),
            in_names=tuple(all_names + ([pid_name] if pid_name else [])),
            out_names=tuple(out_names),
            lowering_input_output_aliases=(),
            sim_require_finite=True,
            sim_require_nnan=True,
            nc=nc,
        )
        return tuple(outs)

    devices = jax.devices()[:B]
    mesh = Mesh(np.asarray(devices), ("core",))
    spec = PartitionSpec("core")
    nouts = len(out_names)
    sharded = jax.jit(
        shard_map(_body, mesh=mesh, in_specs=(spec,) * (n_params + nouts),
                  out_specs=(spec,) * nouts, check_rep=False),
        keep_unused=True)
    sh = NamedSharding(mesh, spec)
    args = []
    for i, name in enumerate(in_names):
        cat = np.concatenate([np.asarray(m[name]) for m in in_maps], axis=0)
        args.append(jax.device_put(cat, sh))
    for av in out_avals:
        z = np.zeros((B * av.shape[0], *av.shape[1:]), av.dtype)
        args.append(jax.device_put(z, sh))
    return sharded, args, out_names, out_avals


def bench(n_iters=30, **inputs):
    import jax
    nc = _get_nc()
    in_maps = make_inputs(**inputs)
    sharded, args, out_names, out_avals = _make_sharded(nc, in_maps)
    out = sharded(*args)
    jax.block_until_ready(out)
    t0 = time.perf_counter()
    for _ in range(n_iters):
        out = sharded(*args)
    jax.block_until_ready(out)
    t1 = time.perf_counter()
    return (t1 - t0) / n_iters * 1e9




